# revision 35
# baseline (speedup 1.0000x reference)
"""Trainium2 Bass kernel for a 2-layer GCN classifier (nn_GCNClassifier).

Reference computation (all f32):
    h1 = relu(adj1 @ x @ W1 + b1) + relu(adj2 @ x @ W1 + b1)   # [8192, 64]
    h2 = relu(adj1 @ h1 @ W2 + b2) + relu(adj2 @ h1 @ W2 + b2) # [8192, 16]

Sharding: 1D row partition of adj1/adj2 across 8 cores (1024 output rows per
core). Each core receives its adjacency row-shard PRE-TRANSPOSED on the host
(adj[rows, :].T, shape [8192, 1024], contiguous, fp16) so the contraction
index lands on the SBUF partition dim and every DMA line is 8KB contiguous.

On-chip layout is feature-major ("transposed space"): aggregates are computed
as aggT[f, m] = sum_k x[k, f] * adjT[k, m] with the tiny feature block as the
stationary matmul operand and the streaming adjacency as the moving operand.
Layer 2 uses associativity: adj @ (h1 @ W2), so only [8192, 16] crosses cores
via AllGather.

Key idea vs a plain two-pass stream: LAYER 2 NEVER TOUCHES HBM for the
adjacency, and 13/16 of LAYER 1's stream is 8-bit. The first 13 k-groups
of each adjacency arrive as fp8-E3M4 ON THE WIRE (4 mantissa bits - the
best 8-bit format for N(0,1) data; measured output rel-err 1.24e-2 vs the
2e-2 gate) straight into a persistent 13MB SBUF cache that BOTH layers
read (mixed-dtype matmul: fp16 stationary x fp8 moving). Only the last 3
k-groups per adjacency stream in fp16 and stay SBUF-resident after L1
(19MB streamed total vs 32). Layer 2 is then a pure PE chew (~15us, 4-way
column-group concurrency) instead of a 65us HBM stream, and the AllGather
trigger - which gates every core - fires at ~92us instead of ~148us.

Other scheduling points:
 - The L1 epilogue (z = W1^T @ agg, relu, branch-sum, g = h1 @ W2) runs in
   fp16 on the z path (fp32 matmuls are 5x slower per instruction), with the
   PSUM->SBUF copies split across ACT and DVE, so every core reaches its
   g-store + AllGather trigger a few us sooner - the collective end is set
   by the slowest core, so this is on the critical path of all cores.
 - One 32KB g store, ONE AllGather, then rotated per-chunk gathers issued in
   the order the in-order PE first needs each chunk. Each core streams its
   OWN k-chunk first in layer 2 (host-rotated layouts + partition-id-offset
   gathers), so own-chunk PE work overlaps the AllGather latency.
 - A tiny warm-up AllGather early in L1 absorbs the ncfw first-collective
   setup (~80us barrier) in the L1 stream shadow.
 - Output is produced feature-major [16, 1024] fp16 (not bf16: same bytes,
   5x less rounding error) and stored with one DMA per m-chunk; the host
   transposes (free).

Engine split: sync and scalar each issue one adjacency stream (one HWDGE
descriptor generator per stream) plus the L2 re-streams and output stores;
DVE does the fp8 shadow casts mid-stream; gpsimd issues all small DMAs
(constants, g bounce, gathers) and the collective.
"""

import numpy as np

import concourse.bacc as bacc
import concourse.bass as bass
import concourse.mybir as mybir
import concourse.tile as tile
from concourse.bass_utils import run_bass_kernel_spmd

N = 8192
IN_DIM, HID_DIM, OUT_DIM = 32, 64, 16
N_CORES = 8
ROWS = N // N_CORES          # 1024 output rows per core
KBLK = 128                   # contraction block (SBUF partition dim)
KMERGE = 4                   # k-blocks fetched per DMA (1MB tiles, 8KB lines)
NKB = N // KBLK              # 64 contraction blocks
NKG = NKB // KMERGE          # 16 merged DMA groups per adjacency
MCHUNK = 512                 # moving free-dim per matmul (PSUM bank limit)
NMC = ROWS // MCHUNK         # 2 m-chunks per core
F32 = mybir.dt.float32
FP16 = mybir.dt.float16
E3M4 = mybir.dt.float8e3
ADJ_DT = FP16
RELU = mybir.ActivationFunctionType.Relu
COPY = mybir.ActivationFunctionType.Copy

STREAM_BUFS = 2              # stream pool (unused when NW8 == NFP8, NSTR == 0)
NW8 = 13                     # k-groups 0..NW8-1 arrive as e3m4 ON THE WIRE
                             # (used by BOTH layers; halves their stream bytes)
NFP8 = 13                    # local k-groups 0..NFP8-1 are e3m4 in layer 2
NSTR = 0                     # k-groups NFP8..NFP8+NSTR-1 re-streamed fp16 in L2
NRES = NKG - NFP8 - NSTR     # trailing k-groups resident in fp16 after L1


def _build_program():
    nc = bacc.Bacc(
        "TRN2", target_bir_lowering=False, debug=False, num_devices=N_CORES
    )
    a1t = nc.dram_tensor("a1t", [NKG, KBLK, KMERGE, ROWS], ADJ_DT, kind="ExternalInput")
    a2t = nc.dram_tensor("a2t", [NKG, KBLK, KMERGE, ROWS], ADJ_DT, kind="ExternalInput")
    a1t8 = nc.dram_tensor("a1t8", [NW8, KBLK, KMERGE, ROWS], E3M4, kind="ExternalInput")
    a2t8 = nc.dram_tensor("a2t8", [NW8, KBLK, KMERGE, ROWS], E3M4, kind="ExternalInput")
    featb = nc.dram_tensor("featb", [KBLK, NKB, IN_DIM], ADJ_DT, kind="ExternalInput")
    w1 = nc.dram_tensor("w1", [IN_DIM, HID_DIM], FP16, kind="ExternalInput")
    b1 = nc.dram_tensor("b1", [HID_DIM, 1], F32, kind="ExternalInput")
    w2 = nc.dram_tensor("w2", [HID_DIM, OUT_DIM], FP16, kind="ExternalInput")
    b2 = nc.dram_tensor("b2", [OUT_DIM, 1], F32, kind="ExternalInput")
    # fp16 output (~1e-4 rounding): halves the store bytes vs f32; the host
    # casts back to f32 (values < 3.1e4 << fp16 max)
    out = nc.dram_tensor("out", [OUT_DIM, ROWS], FP16, kind="ExternalOutput")

    with tile.TileContext(nc) as tc:
        _kernel_body(nc, tc, a1t, a2t, a1t8, a2t8, featb, w1, b1, w2, b2, out)
    nc.compile()
    return nc


def _kernel_body(nc, tc, a1t, a2t, a1t8, a2t8, featb, w1, b1, w2, b2, out):
    with (
        tc.tile_pool(name="const", bufs=1) as constp,
        tc.tile_pool(name="cach8", bufs=1) as cach8p,
        tc.tile_pool(name="res", bufs=1) as resp,
        tc.tile_pool(name="adj", bufs=STREAM_BUFS) as adjp,
        tc.tile_pool(name="work", bufs=1) as workp,
        tc.tile_pool(name="psum", bufs=1, space="PSUM") as psp,
        tc.tile_pool(name="dram", bufs=1, space="DRAM") as dramp,
    ):
        # --- tiny warm-up AllGather first: wakes the ncfw/CC path during the
        # L1 stream so the real inter-layer exchange doesn't pay first-use
        # setup, and acts as an early cross-core alignment barrier
        warm_sb = constp.tile([1, N_CORES], F32)
        nc.gpsimd.memset(warm_sb[:], 0.0)
        warm_in = dramp.tile([1, N_CORES], F32)
        warm_out = dramp.tile([N_CORES, N_CORES], F32, addr_space="Shared")
        nc.gpsimd.dma_start(warm_in[:], warm_sb[:])
        nc.gpsimd.collective_compute(
            "AllGather",
            mybir.AluOpType.bypass,
            replica_groups=[list(range(N_CORES))],
            ins=[warm_in.opt()],
            outs=[warm_out.opt()],
        )
        # --- constants on gpsimd (SWDGE) so the sync/scalar HWDGE queues
        # start the adjacency stream immediately
        xb = constp.tile([KBLK, NKB, IN_DIM], ADJ_DT)   # features, k-blocked
        nc.gpsimd.dma_start(xb[:], featb[:])
        # W1 (fp16) replicated at partition offsets 0/32/64/96 so each
        # z-matmul can read its aggregate chunk straight out of the packed
        # accumulator copy (PE row group = chunk's partition offset)
        w1_sb = constp.tile([128, HID_DIM], FP16)
        for g in range(4):
            nc.gpsimd.dma_start(w1_sb[g * 32:g * 32 + IN_DIM, :], w1[:])
        b1_sb = constp.tile([HID_DIM, 1], F32)
        nc.gpsimd.dma_start(b1_sb[:], b1[:])
        w2_sb = constp.tile([HID_DIM, OUT_DIM], FP16)
        nc.gpsimd.dma_start(w2_sb[:], w2[:])
        b2_sb = constp.tile([OUT_DIM, 1], F32)
        nc.gpsimd.dma_start(b2_sb[:], b2[:])

        # persistent L2 sources: e3m4 shadows for k-groups 0..NFP8-1,
        # fp16 residents for the trailing NRES groups
        c8 = {}
        for ai in range(2):
            for kg in range(NFP8):
                c8[(ai, kg)] = cach8p.tile(
                    [KBLK, KMERGE, ROWS], E3M4, name=f"c8_{ai}_{kg}"
                )
        res = {}
        for ai in range(2):
            for kg in range(NKG - NRES, NKG):
                res[(ai, kg)] = resp.tile(
                    [KBLK, KMERGE, ROWS], ADJ_DT, name=f"res{ai}_{kg}"
                )

        adj_drams = (a1t, a2t)
        adj8_drams = (a1t8, a2t8)
        dma_engines = (nc.sync, nc.scalar)   # one HWDGE generator per adjacency
        l2src = {}                           # (ai, kg) -> SBUF tile for L2
        deferred_casts = []                  # adj2 fp8 casts, run in AG window

        # --- layer 1: stream everything once, fp8-shadow in the DVE shadow.
        # Two accumulators: acc1 takes k-groups 0..NKG-3 and is folded into
        # z (phase a, below) while the last two groups stream into acc1t;
        # only acc1t's small contribution sits on the post-stream path.
        KB_SPLIT = (NKG - 2) * KMERGE
        acc1 = psp.tile([128, MCHUNK], F32, tag="accm", name="l1m")
        acc1t = psp.tile([128, MCHUNK], F32, tag="accmt", name="l1mt")
        asb = workp.tile([128, MCHUNK], FP16)
        asbt = workp.tile([128, MCHUNK], FP16, name="asbt")
        zfull = [
            psp.tile([128, MCHUNK], F32, tag=f"zz{mc}", name=f"z{mc}")
            for mc in range(NMC)
        ]
        w1s = w1_sb

        def _z_phase(acc, sb, first):
            # z += W1^T @ (fp16 copy of acc chunk); the two branches go to
            # disjoint PE column groups (zfull halves) and run concurrently.
            # Phase a copies all ride the DVE: an ACT copy there would block
            # the scalar queue ahead of the adj2 tail-group DMA issues.
            for mc in range(NMC):
                for ai in range(2):
                    off = (ai * NMC + mc) * 32
                    if first or ai == 0:
                        nc.vector.tensor_copy(
                            sb[off:off + IN_DIM, :], acc[off:off + IN_DIM, :]
                        )
                    else:
                        nc.scalar.activation(
                            sb[off:off + IN_DIM, :], acc[off:off + IN_DIM, :],
                            COPY,
                        )
                    nc.tensor.matmul(
                        zfull[mc][ai * 64:(ai + 1) * 64, :],
                        w1s[off:off + IN_DIM, :],
                        sb[off:off + IN_DIM, :],
                        start=first,
                        stop=not first,
                        tile_position=(off, ai * 64),
                        skip_group_check=True,
                    )

        for kg in range(NKG):
            for ai in range(2):
                eng = dma_engines[ai]
                if kg < NW8:
                    # e3m4 on the wire, straight into the L2 cache tile
                    # (half the bytes; both layers read it; no cast)
                    at = c8[(ai, kg)]
                    src = adj8_drams[ai][kg]
                elif kg >= NKG - NRES:
                    at = res[(ai, kg)]
                    src = adj_drams[ai][kg]
                else:
                    at = adjp.tile(
                        [KBLK, KMERGE, ROWS], ADJ_DT, tag="adj",
                        name=f"l1_adj{ai}_{kg}",
                    )
                    src = adj_drams[ai][kg]
                if kg == 0 or kg == NKG - 1:
                    # split the first transfer (data flows before the full
                    # descriptor set is generated) and the last one
                    # (PE epilogue starts on the first half earlier)
                    half = KMERGE // 2
                    eng.dma_start(at[:, :half, :], src[:, :half, :])
                    eng.dma_start(at[:, half:, :], src[:, half:, :])
                else:
                    eng.dma_start(at[:], src)
                for t in range(KMERGE):
                    kb = kg * KMERGE + t
                    tail = kb >= KB_SPLIT
                    acc = acc1t if tail else acc1
                    for mc in range(NMC):
                        off = (ai * NMC + mc) * 32
                        nc.tensor.matmul(
                            acc[off:off + IN_DIM, :],
                            xb[:, kb, :],
                            at[:, t, mc * MCHUNK:(mc + 1) * MCHUNK],
                            start=(kb == 0 or kb == KB_SPLIT),
                            stop=(kb == KB_SPLIT - 1 or kb == NKB - 1),
                            tile_position=(0, off),
                        )
                if kg < NW8:
                    l2src[(ai, kg)] = at          # already the e3m4 tile
                elif kg < NFP8:
                    # adj1 groups shadow-cast inline on the mid-stream-idle
                    # DVE; adj2 casts emitted late (the dep-driven scheduler
                    # runs them whenever the DVE is free anyway)
                    # (gpsimd's Q7 has no fast fp8 path - 15.8us/cast)
                    if ai == 0:
                        nc.vector.tensor_copy(c8[(ai, kg)][:], at[:])
                    else:
                        deferred_casts.append((c8[(ai, kg)], at))
                    l2src[(ai, kg)] = c8[(ai, kg)]
                elif kg >= NKG - NRES:
                    l2src[(ai, kg)] = at
            if kg == NKG - 3:
                # phase a: fold the bulk aggregate into z in the stream
                # shadow (emitted here so the in-order PE runs it before
                # the tail groups' matmuls)
                _z_phase(acc1, asb, first=True)

        # --- L2 fp16 re-streams: issued right behind the L1 stream on the
        # same HWDGE queues; they land during the epilogue + AllGather wait
        # when the DMA path is otherwise idle (second read of 3MB/adjacency)
        for kg in range(NFP8, NFP8 + NSTR):
            for ai in range(2):
                at = adjp.tile(
                    [KBLK, KMERGE, ROWS], ADJ_DT, tag="adj",
                    name=f"l2s_adj{ai}_{kg}",
                )
                dma_engines[ai].dma_start(at[:], adj_drams[ai][kg])
                l2src[(ai, kg)] = at

        # --- L1 epilogue tail: fold acc1t into z (phase b), then per
        # m-chunk relu both branches (ACT / DVE in parallel), branch-sum,
        # 4 g-block matmuls, cast-copy, store half + collective.
        _z_phase(acc1t, asbt, first=False)
        h1_parts = [
            workp.tile([HID_DIM, ROWS], FP16, name=f"h1p{ai}") for ai in range(2)
        ]
        h1T = workp.tile([HID_DIM, ROWS], FP16)
        nloc = ROWS // KBLK                              # 8 local k-blocks
        nhalf = nloc // NMC                              # g blocks per half
        g_ps = psp.tile([KBLK, nloc, OUT_DIM], F32, tag="gg")
        g_sb = workp.tile([KBLK, nloc, OUT_DIM], ADJ_DT)
        for mc in range(NMC):
            csl = slice(mc * MCHUNK, (mc + 1) * MCHUNK)
            nc.scalar.activation(
                h1_parts[0][:, csl], zfull[mc][0:HID_DIM, :], RELU, bias=b1_sb[:]
            )
            # second branch's relu on DVE (fused z+b then max 0)
            nc.vector.tensor_scalar(
                h1_parts[1][:, csl],
                zfull[mc][64:64 + HID_DIM, :],
                b1_sb[:],
                0.0,
                mybir.AluOpType.add,
                mybir.AluOpType.max,
            )
            nc.vector.tensor_add(
                h1T[:, csl], h1_parts[0][:, csl], h1_parts[1][:, csl]
            )
            hsl = slice(mc * nhalf, (mc + 1) * nhalf)
            for i in range(mc * nhalf, (mc + 1) * nhalf):
                nc.tensor.matmul(
                    g_ps[:, i, :],
                    h1T[:, i * KBLK:(i + 1) * KBLK],
                    w2_sb[:],
                    start=True,
                    stop=True,
                )
            nc.vector.tensor_copy(g_sb[:, hsl, :], g_ps[:, hsl, :])

        # two 16KB store halves (each issued as soon as its m-chunk's g is
        # ready; gpsimd SWDGE - the HWDGE queues carry the L2 re-streams),
        # then the AllGather
        g_loc = dramp.tile([KBLK, nloc * OUT_DIM], ADJ_DT)
        gflat = g_sb[:].rearrange("p j o -> p (j o)")
        half = nloc * OUT_DIM // 2
        nc.gpsimd.dma_start(g_loc[:, :half], gflat[:, :half])
        nc.gpsimd.dma_start(g_loc[:, half:], gflat[:, half:])
        g_cat = dramp.tile([N_CORES * KBLK, nloc * OUT_DIM], ADJ_DT,
                           addr_space="Shared")
        nc.gpsimd.collective_compute(
            "AllGather",
            mybir.AluOpType.bypass,
            replica_groups=[list(range(N_CORES))],
            ins=[g_loc.opt()],
            outs=[g_cat.opt()],
        )
        # adj2's fp8 shadow casts, deferred into the AllGather wait: the DVE
        # is idle here, the source stream tiles' pool slots are never reused
        # (only kg >= NFP8 slots recycle into the re-streams), and the first
        # L2 consumer of these tiles runs well after the collective. Emitted
        # in L2 consumption order (kg ascending).
        for c8t, srct in deferred_casts:
            nc.vector.tensor_copy(c8t[:], srct[:])

        # Remote g chunks, gathered in per-core rotated order: local k-chunk
        # j covers global chunk (pid + j) mod 8, so every core consumes its
        # OWN chunk first (lhs straight from g_sb, no AllGather dep) and
        # gathers the rest in the order the in-order PE needs them.
        gb2 = constp.tile([KBLK, NKB - nloc, OUT_DIM], ADJ_DT)
        geng = (nc.sync, nc.scalar, nc.gpsimd)
        pids = [e.partition_id() for e in geng]
        for j in range(1, N_CORES):
            # split gathers across the 3 DMA issuers (all idle here) so the
            # first chunks the in-order PE needs land concurrently
            eng = geng[(j - 1) % 3]
            q = (pids[(j - 1) % 3] + j) & (N_CORES - 1)
            eng.dma_start(
                gb2[:, (j - 1) * nloc:j * nloc, :],
                g_cat[bass.ds(q * KBLK, KBLK), :]
                .rearrange("p (j2 o) -> p j2 o", j2=nloc),
            )

        def l2_lhs(kb):
            return g_sb[:, kb, :] if kb < nloc else gb2[:, kb - nloc, :]

        # --- layer 2: pure SBUF chew, zero HBM. Inner order (t, ai, mc)
        # spreads 4 consecutive matmuls over the 4 PE column groups for
        # concurrent execution.
        acc2 = psp.tile([128, MCHUNK], F32, tag="accm", name="l2m")
        for kg in range(NKG):
            for t in range(KMERGE):
                kb = kg * KMERGE + t
                lhs = l2_lhs(kb)
                for ai in range(2):
                    src = l2src[(ai, kg)]
                    for mc in range(NMC):
                        off = (ai * NMC + mc) * 32
                        nc.tensor.matmul(
                            acc2[off:off + OUT_DIM, :],
                            lhs,
                            src[:, t, mc * MCHUNK:(mc + 1) * MCHUNK],
                            start=(kb == 0),
                            stop=(kb == NKB - 1),
                            tile_position=(0, off),
                        )

        # h2T = relu(agg2T + b2) summed over branches, stored feature-major
        # with a single DMA per m-chunk (host transposes for free)
        h2_parts = [
            workp.tile([OUT_DIM, ROWS], FP16, name=f"h2p{ai}") for ai in range(2)
        ]
        h2T = workp.tile([OUT_DIM, ROWS], FP16)
        oeng = (nc.sync, nc.scalar)
        for mc in range(NMC):
            sl = slice(mc * MCHUNK, (mc + 1) * MCHUNK)
            # branch relus on two engines in parallel (ACT activation /
            # DVE fused add+max), branch sum on the idle gpsimd engine
            off0 = mc * 32
            off1 = (NMC + mc) * 32
            nc.scalar.activation(
                h2_parts[0][:, sl],
                acc2[off0:off0 + OUT_DIM, :],
                RELU,
                bias=b2_sb[:],
            )
            nc.vector.tensor_scalar(
                h2_parts[1][:, sl],
                acc2[off1:off1 + OUT_DIM, :],
                b2_sb[:],
                0.0,
                mybir.AluOpType.add,
                mybir.AluOpType.max,
            )
            # branch-sum on DVE right behind its own tensor_scalar (gpsimd's
            # Q7 pays a library swap + 1.15us for the same add)
            nc.vector.tensor_add(
                h2T[:, sl], h2_parts[0][:, sl], h2_parts[1][:, sl]
            )
            oeng[mc % 2].dma_start(out[:, sl], h2T[:, sl])


_NC_CACHE = None


def _get_nc():
    global _NC_CACHE
    if _NC_CACHE is None:
        _NC_CACHE = _build_program()
    return _NC_CACHE


def _shard_inputs(inputs):
    wire_np = np.float16
    adj1 = np.asarray(inputs["adj1"], dtype=np.float32)
    adj2 = np.asarray(inputs["adj2"], dtype=np.float32)
    feat = np.asarray(inputs["features"], dtype=np.float32)
    featb0 = np.ascontiguousarray(
        feat.reshape(NKB, KBLK, IN_DIM).swapaxes(0, 1)
    ).astype(wire_np)
    w1 = np.ascontiguousarray(inputs["W1"]).astype(np.float16)
    b1 = np.ascontiguousarray(inputs["b1"], dtype=np.float32).reshape(HID_DIM, 1)
    w2 = np.ascontiguousarray(inputs["W2"]).astype(np.float16)
    b2 = np.ascontiguousarray(inputs["b2"], dtype=np.float32).reshape(OUT_DIM, 1)
    in_maps = []
    for c in range(N_CORES):
        rows = slice(c * ROWS, (c + 1) * ROWS)
        # per-core rotation: stream this core's own k-chunk (blocks 8c..8c+7)
        # first; matches the kernel's (pid + j) mod 8 gather order
        featb = np.ascontiguousarray(np.roll(featb0, -c * ROWS // KBLK, axis=1))

        # blocked-transposed: [kg, p, t, m] = adj[c*ROWS + m, kg*KM*128 + t*128 + p]
        def blockT(a):
            blocked = (
                a[rows, :]
                .reshape(ROWS, NKG, KMERGE, KBLK)
                .transpose(1, 3, 2, 0)
                .astype(wire_np)
            )
            ngrp_per_core = ROWS // (KMERGE * KBLK)      # groups per chunk
            return np.ascontiguousarray(
                np.roll(blocked, -c * ngrp_per_core, axis=0)
            )
        import ml_dtypes
        b1t, b2t = blockT(adj1), blockT(adj2)
        in_maps.append({
            "a1t": b1t,
            "a2t": b2t,
            "a1t8": b1t[:NW8].astype(ml_dtypes.float8_e3m4),
            "a2t8": b2t[:NW8].astype(ml_dtypes.float8_e3m4),
            "featb": featb,
            "w1": w1,
            "b1": b1,
            "w2": w2,
            "b2": b2,
        })
    return in_maps


def _ensure_ntff_shim():
    # bass_utils' axon trace path imports antenv.axon_hooks, which this agent
    # image lacks; stub it so a stray BASS_TRACE=1 env can't crash the run.
    import sys as _sys
    try:
        import antenv.axon_hooks  # noqa: F401
    except ImportError:
        import types as _types
        mod = _types.ModuleType("antenv.axon_hooks")
        _state = {"hook": None}
        mod.set_axon_ntff_profile_hook = lambda h: _state.__setitem__("hook", h)
        mod.get_axon_ntff_profile_hook = lambda: _state["hook"]
        _sys.modules["antenv.axon_hooks"] = mod


def _run(inputs, trace=False, trace_cores=None, stitch_traces=False):
    _ensure_ntff_shim()
    nc = _get_nc()
    in_maps = _shard_inputs(inputs)
    res = run_bass_kernel_spmd(
        nc,
        in_maps,
        core_ids=list(range(N_CORES)),
        trace=trace,
        trace_cores=trace_cores,
        stitch_traces=stitch_traces,
    )
    full = np.concatenate(
        [res.results[c]["out"].T for c in range(N_CORES)], axis=0
    ).astype(np.float32)
    return full, res


def kernel(**inputs):
    full, _ = _run(inputs, trace=False)
    return full
